# revision 21
# baseline (speedup 1.0000x reference)
"""Trainium2 Bass kernel for ByteLatentEncoder topk_mean_pooling (segment top-4 mean).

Problem: h [8, 4096, 512] f32, patch_ids [8, 4096] int64 (sorted per row,
values in [0, 1024)).  Output [8, 1024, 512]: per (batch, patch, channel),
mean of the top-min(4, count) segment values with the reference's knockout
semantics (exact float ties collapse; exhausted ranks contribute -1e9).

Design (data-parallel over batch, one NeuronCore per row):
  - h is host-staged to bf16 [4108, 512] (12 zero pad rows; row 4096 is the
    window target for empty slots / count-0 patches).
  - Patches are classed by count c; windows are fetched with per-q indirect
    window DMAs: ONE contiguous W-row descriptor per patch (partition-prefix
    trimmed), keeping Q7 descriptor-gen cheap:
      A4: c==4 (+ all c<=4 exact-tie patches), W=4, plain sum, 1/c scale.
      A3: c==3 W=3; A2: c==2 W=2; A1: c<=1 W=1 (count-0 reads a zero row).
      B: 5<=c<=6 (<=256, overflow spills into C): a 4-block + a 2-pair
         (fetched separately so the block joins the uniform block array).
      C: 7<=c<=8 (+B overflow), W=8 = two 4-blocks, <=128 patches.
      D: 9<=c<=12, W=12 = three 4-blocks, <=128 patches.
  - All B/C/D 4-blocks live in ONE [P, 7, 4, 512] bf16 array (uniform 2KB
    block stride), so a single 10-instruction sort4 network (J=7 APs, DVE
    bf16 2x mode) sorts every block at once.  Foreign slots (w >= c: B pair
    w5, C w5-7, D w9-11) are pre-killed on the ACT engine via
    Identity(x*m + a) with per-partition 0/1 and 0/-1e9 scalars.
  - Top-4 per patch: bitonic 4-merges of sorted blocks (C and D's first
    merge share J=2 instructions); D re-sorts its bitonic output (4 CE)
    before merging the third block.  Ties need no handling here
    (multiplicity top-4 == reference for c>=5).
  - ACT applies (sum * scale) with bf16->f32 cast.  The few c<=4 exact-tie
    patches (where the reference sums -1e9 knockout terms) sit at the front
    of A4 q0 and get a host-baked additive f32 fix plane.
  - Output: B/C/D rows scatter via 4 early indirect DMAs; the 7 A-class
    result planes go through one dma_scatter_add whose descriptors are
    prepared early (prepare_only) and triggered once the last ACT lands,
    into a zero-initialized out[1026] (rows 1024/1025 catch empty slots;
    the host slices [:1024]).
"""

from contextlib import ExitStack

import numpy as np
import ml_dtypes

import concourse.bacc as bacc
import concourse.bass as bass
import concourse.mybir as mybir
import concourse.tile as tile
from concourse.bass_utils import run_bass_kernel_spmd

P = 128
SEQ = 4096
DIM = 512
NPATCH = 1024
K = 4
NEG = -1.0e9
OOB = 1 << 20

ZROW = SEQ           # zero row for empty/count-0 windows
NH = SEQ + 12        # 12 pad rows so any window read stays in bounds

BF16 = ml_dtypes.bfloat16

NQ = dict(a4=2, a3=2, a2=2, a1=1, b=2, c=1, d=1)
WW = dict(a4=4, a3=3, a2=2, a1=1, b=4, c=8, d=12)   # gathered rows per slot
# result columns (scatter planes); A-cols first (they go via scatter_add)
COLS = ["a4q0", "a4q1", "a3q0", "a3q1", "a2q0", "a2q1", "a1", "bq0", "bq1",
        "c", "d"]
NACOL = 7
NCOL = len(COLS)
# gather columns: the result cols double as window offsets, plus B pairs
GCOLS = COLS + ["bpq0", "bpq1"]
NGCOL = len(GCOLS)
# block-plane layout in the joint block array: j -> (class, q)
BLKJ = [("b", 0), ("b", 1), ("c", 0), None, ("d", 0), None, None]
# mask planes: (class, q, w in window)
MASKS = [("b", 0, 5), ("b", 1, 5), ("c", 0, 5), ("c", 0, 6), ("c", 0, 7),
         ("d", 0, 9), ("d", 0, 10), ("d", 0, 11)]


def _find_ties(h_row, starts, counts, plist):
    out = set()
    for p in plist:
        c = int(counts[p])
        if c < 2:
            continue
        seg = h_row[starts[p]:starts[p] + c]
        s = np.sort(seg, axis=0)
        if (s[1:] == s[:-1]).any():
            out.add(p)
    return out


def build_row_tables(h_row, pid_row):
    starts = np.searchsorted(pid_row, np.arange(NPATCH + 1)).astype(np.int64)
    counts = np.diff(starts)
    starts = starts[:-1]
    assert counts.max() <= 12, counts.max()

    by = {k: [] for k in NQ}
    for p in range(NPATCH):
        c = counts[p]
        if c == 4:
            by["a4"].append(p)
        elif c == 3:
            by["a3"].append(p)
        elif c == 2:
            by["a2"].append(p)
        elif c <= 1:
            by["a1"].append(p)
        elif c <= 6:
            by["b"].append(p)
        elif c <= 8:
            by["c"].append(p)
        else:
            by["d"].append(p)

    ties = _find_ties(h_row, starts, counts,
                      by["a4"] + by["a3"] + by["a2"])
    if ties:
        for k in ("a3", "a2"):
            by[k] = [p for p in by[k] if p not in ties]
        by["a4"] = sorted(ties) + [p for p in by["a4"] if p not in ties]

    if len(by["b"]) > NQ["b"] * P:
        by["c"] = by["c"] + by["b"][NQ["b"] * P:]
        by["b"] = by["b"][:NQ["b"] * P]
    for k in NQ:
        assert len(by[k]) <= NQ[k] * P, (k, len(by[k]))

    # fix plane: expected minus what the device computes for tie patches
    fixpl = np.zeros((P, DIM), np.float32)
    for i, p in enumerate(sorted(ties)):
        c = int(counts[p])
        win = h_row[starts[p]:starts[p] + 4]
        if win.shape[0] < 4:
            win = np.concatenate(
                [win, np.zeros((4 - win.shape[0], DIM), np.float32)], 0)
        plain = win.sum(axis=0) / c
        seg = h_row[starts[p]:starts[p] + c]
        ref = np.zeros(DIM, np.float32)
        for ch in range(DIM):
            u = np.unique(seg[:, ch])
            nd = len(u)
            ref[ch] = (u[::-1][:c].sum() + max(0, c - nd) * NEG) / c
        fixpl[i] = ref - plain

    woff = np.full((P, NGCOL), ZROW, np.int32)
    srow = np.full((P, NCOL), OOB, np.int32)
    nrow = np.zeros(NGCOL, np.int32)
    sca4 = np.full((P, NQ["a4"]), 0.25, np.float32)
    m01 = np.ones((P, len(MASKS)), np.float32)
    madd = np.zeros((P, len(MASKS)), np.float32)

    def col_id(cls, q):
        return COLS.index((cls + f"q{q}") if NQ[cls] > 1 else cls)

    assert len(by["d"]) <= 32, len(by["d"])
    for cls in NQ:
        for q in range(NQ[cls]):
            cid = col_id(cls, q)
            for p in range(P):
                s = q * P + p
                if s >= len(by[cls]):
                    continue
                pat = by[cls][s]
                c = int(counts[pat])
                if cls == "d":
                    # stacked: block b of patch p -> partition 32b+p
                    for b in range(3):
                        woff[32 * b + p, cid] = starts[pat] + 4 * b
                else:
                    woff[p, cid] = starts[pat] if c > 0 else ZROW
                if cls == "b":
                    woff[p, NCOL + q] = starts[pat] + 4    # pair window
                    nrow[NCOL + q] = p + 1
                srow[p, cid] = pat
                nrow[cid] = p + 1
                if cls == "a4":
                    sca4[p, q] = 1.0 / c
    for mi, (cls, q, w) in enumerate(MASKS):
        if cls == "d":
            # stacked: the mask ACT runs on partitions [64, 96) (block 2)
            # with per-partition scalars from tf rows 64+p, plane w%4
            for p in range(32):
                s = q * P + p
                if s >= len(by[cls]) or w >= counts[by[cls][s]]:
                    m01[64 + p, mi] = 0.0
                    madd[64 + p, mi] = NEG
            continue
        for p in range(P):
            s = q * P + p
            if s >= len(by[cls]) or w >= counts[by[cls][s]]:
                m01[p, mi] = 0.0
                madd[p, mi] = NEG

    itab32 = np.concatenate([woff, srow], axis=1).astype(np.int32)
    ftab = np.concatenate([sca4, m01, madd, fixpl], axis=1).astype(np.float32)
    return dict(itab32=np.ascontiguousarray(itab32),
                ftab=np.ascontiguousarray(ftab)), nrow, srow


def prepare(h, patch_ids):
    h = np.asarray(h, np.float32)
    pid = np.asarray(patch_ids)
    in_maps = []
    nrows = []
    srows = []
    for b in range(h.shape[0]):
        t, nrow, srow = build_row_tables(h[b], pid[b])
        hb = np.concatenate(
            [h[b], np.zeros((NH - SEQ, DIM), np.float32)], axis=0).astype(BF16)
        in_maps.append(dict(hb=np.ascontiguousarray(hb), **t))
        nrows.append(nrow)
        srows.append(srow)
    nrow = np.maximum.reduce(nrows)
    nrow = np.maximum(nrow, 2)
    return in_maps, nrow.tolist(), srows


# ---------------------------------------------------------------------------
# Device kernel
# ---------------------------------------------------------------------------

class Plane:
    def __init__(self, t, off, jstride=0):
        self.t = t
        self.off = off
        self.jstride = jstride

    def ap(self, J=1):
        base = self.t[:]
        if J == 1:
            return bass.AP(base.tensor, base.offset + self.off,
                           [base.ap[0], [1, DIM]])
        return bass.AP(base.tensor, base.offset + self.off,
                       [base.ap[0], [self.jstride, J], [1, DIM]])


def emit_sort4(nc, X, T, J, xstride, tstride, xoff=0, toff=0):
    """Sort each 4-block (desc) across J j-planes; T is scratch.
    Returns sorted plane handles [A0, A1, A2, A3] (at j=0 offsets)."""
    mx = mybir.AluOpType.max
    mn = mybir.AluOpType.min
    tt = nc.vector.tensor_tensor
    x0, x1, x2, x3 = (Plane(X, xoff + w * DIM, xstride) for w in range(4))
    t0, t1, t2, t3 = (Plane(T, toff + w * DIM, tstride) for w in range(4))
    tt(t0.ap(J), x0.ap(J), x1.ap(J), op=mx)
    tt(t1.ap(J), x0.ap(J), x1.ap(J), op=mn)
    tt(t2.ap(J), x2.ap(J), x3.ap(J), op=mx)
    tt(t3.ap(J), x2.ap(J), x3.ap(J), op=mn)
    tt(x0.ap(J), t0.ap(J), t2.ap(J), op=mx)   # A0
    tt(x1.ap(J), t0.ap(J), t2.ap(J), op=mn)   # u
    tt(x3.ap(J), t1.ap(J), t3.ap(J), op=mn)   # A3
    tt(x2.ap(J), t1.ap(J), t3.ap(J), op=mx)   # v
    tt(t0.ap(J), x1.ap(J), x2.ap(J), op=mx)   # A1
    tt(t1.ap(J), x1.ap(J), x2.ap(J), op=mn)   # A2
    return [x0, t0, t1, x3]


def emit_sort4_contig(nc, X, T, J, xstride, tstride):
    """Like emit_sort4 but leaves the sorted block contiguous in X w0..w3
    (one extra plane copy, placed on the ACT engine)."""
    mx = mybir.AluOpType.max
    mn = mybir.AluOpType.min
    tt = nc.vector.tensor_tensor
    x0, x1, x2, x3 = (Plane(X, w * DIM, xstride) for w in range(4))
    t0, t1, t2, t3 = (Plane(T, w * DIM, tstride) for w in range(4))
    tt(t0.ap(J), x0.ap(J), x1.ap(J), op=mx)
    tt(t1.ap(J), x0.ap(J), x1.ap(J), op=mn)
    tt(t2.ap(J), x2.ap(J), x3.ap(J), op=mx)
    tt(t3.ap(J), x2.ap(J), x3.ap(J), op=mn)
    tt(x0.ap(J), t0.ap(J), t2.ap(J), op=mx)   # A0
    tt(x1.ap(J), t0.ap(J), t2.ap(J), op=mn)   # u
    tt(x3.ap(J), t1.ap(J), t3.ap(J), op=mn)   # A3
    tt(x2.ap(J), t1.ap(J), t3.ap(J), op=mx)   # v
    tt(t0.ap(J), x1.ap(J), x2.ap(J), op=mx)   # A1 -> scratch
    tt(x2.ap(J), x1.ap(J), x2.ap(J), op=mn)   # A2 in place
    nc.scalar.activation(x1.ap(J), t0.ap(J),
                         mybir.ActivationFunctionType.Copy)  # A1 -> x1


class PPlane:
    """A plane on a partition slice [p0, p0+n) of a tile."""

    def __init__(self, t, p0, n, off):
        self.t = t
        self.p0 = p0
        self.n = n
        self.off = off

    def ap(self, J=1):
        assert J == 1
        s = self.t[self.p0:self.p0 + self.n, :]
        return bass.AP(s.tensor, s.offset + self.off, [s.ap[0], [1, DIM]])


def emit_merge4(nc, dst, a, b, J=1):
    mx = mybir.AluOpType.max
    for i in range(4):
        nc.vector.tensor_tensor(dst[i].ap(J), a[i].ap(J), b[3 - i].ap(J), op=mx)


def emit_bitonic_sort4(nc, dst, c, J=1):
    mx = mybir.AluOpType.max
    mn = mybir.AluOpType.min
    tt = nc.vector.tensor_tensor
    d0, d1, d2, d3 = dst
    tt(d0.ap(J), c[0].ap(J), c[2].ap(J), op=mx)
    tt(d2.ap(J), c[0].ap(J), c[2].ap(J), op=mn)
    tt(d1.ap(J), c[1].ap(J), c[3].ap(J), op=mx)
    tt(d3.ap(J), c[1].ap(J), c[3].ap(J), op=mn)
    tt(c[0].ap(J), d0.ap(J), d1.ap(J), op=mx)
    tt(c[1].ap(J), d0.ap(J), d1.ap(J), op=mn)
    tt(c[2].ap(J), d2.ap(J), d3.ap(J), op=mx)
    tt(c[3].ap(J), d2.ap(J), d3.ap(J), op=mn)
    return c


def emit_sum4(nc, out_ap, planes, J, s0, s1):
    add = mybir.AluOpType.add
    tt = nc.vector.tensor_tensor
    tt(s0.ap(J), planes[0].ap(J), planes[1].ap(J), op=add)
    tt(s1.ap(J), planes[2].ap(J), planes[3].ap(J), op=add)
    tt(out_ap, s0.ap(J), s1.ap(J), op=add)


def build_kernel(ctx, tc, nrow):
    nc = tc.nc
    dt = mybir.dt
    bf = dt.bfloat16
    Copy = mybir.ActivationFunctionType.Copy
    Ident = mybir.ActivationFunctionType.Identity
    NFCOL = NQ["a4"] + 2 * len(MASKS) + DIM

    in_aps = {}
    specs = dict(
        hb=((NH, DIM), bf),
        itab32=((P, NGCOL + NCOL), dt.int32),
        ftab=((P, NFCOL), dt.float32),
    )
    for name, (shape, dtype) in specs.items():
        in_aps[name] = nc.dram_tensor(name, list(shape), dtype,
                                      kind="ExternalInput").ap()
    # one DRAM tensor per scatter column: disjoint tensors keep Tile from
    # serializing the scatters on whole-tensor WAW (DMA-completion waits)
    out_aps = [nc.dram_tensor(f"out{c}", [NPATCH, DIM], dt.float32,
                              kind="ExternalOutput").ap()
               for c in range(NCOL)]

    tabs = ctx.enter_context(tc.tile_pool(name="tabs", bufs=1))
    big = ctx.enter_context(tc.tile_pool(name="big", bufs=1))

    t32 = tabs.tile([P, NGCOL + NCOL], dt.int32, tag="t32")
    tf = tabs.tile([P, NFCOL], dt.float32, tag="tf")
    nc.sync.dma_start(t32[:], in_aps["itab32"][:])
    nc.sync.dma_start(tf[:], in_aps["ftab"][:])

    # per-class block arrays (separate tiles so tile-granular deps don't
    # chain one class's sort behind another's gather/mask)
    WB2 = big.tile([P, 2 * 4 * DIM], bf, tag="wb2")
    WC2 = big.tile([P, 2 * 4 * DIM], bf, tag="wc2")
    WD1 = big.tile([P, 4 * DIM], bf, tag="wd1")
    WDP = big.tile([P, 8 * DIM], bf, tag="wdp")
    WPR = big.tile([P, NQ["b"] * 2 * DIM], bf, tag="wpr")
    WA = {}
    for cls in ("a4", "a3", "a2", "a1"):
        WA[cls] = big.tile([P, NQ[cls] * WW[cls] * DIM], bf, tag="w" + cls,
                           name="w" + cls)
    TSB = big.tile([P, 2 * 4 * DIM], bf, tag="tsb")
    TSC = big.tile([P, 2 * 4 * DIM], bf, tag="tsc")
    TSD = big.tile([P, 4 * DIM], bf, tag="tsd")
    ME = big.tile([P, 2 * 4 * DIM], bf, tag="me")      # C/D merge planes
    M2 = big.tile([P, 8 * DIM], bf, tag="m2")          # D bitonic + final
    SUM = big.tile([P, NCOL * DIM], bf, tag="sum")
    RS = big.tile([P, NCOL * DIM], dt.float32, tag="rs")

    def gather(dst, dst_off, w, gcid):
        # always 128 partitions: few-partition indirect DMAs collapse onto
        # one SDMA engine (trace: a 31-desc gather ran 21us serial)
        base = dst[:]
        ap = bass.AP(base.tensor, base.offset + dst_off,
                     [[base.ap[0][0], P], [1, w * DIM]])
        nc.gpsimd.indirect_dma_start(
            out=ap, out_offset=None, in_=in_aps["hb"][:],
            in_offset=bass.IndirectOffsetOnAxis(ap=t32[:, gcid:gcid + 1],
                                                axis=0))

    # issue order = DMA arrival order = DVE consumption order
    gather(WB2, 0 * 4 * DIM, 4, GCOLS.index("bq0"))
    gather(WB2, 1 * 4 * DIM, 4, GCOLS.index("bq1"))
    gather(WPR, 0, 2, GCOLS.index("bpq0"))
    gather(WPR, 2 * DIM, 2, GCOLS.index("bpq1"))
    gather(WC2, 0, 8, GCOLS.index("c"))
    gather(WD1, 0, 4, GCOLS.index("d"))
    for cls in ("a4", "a3", "a2", "a1"):
        for q in range(NQ[cls]):
            gather(WA[cls], q * WW[cls] * DIM, WW[cls],
                   GCOLS.index((cls + f"q{q}") if NQ[cls] > 1 else cls))

    def sum_ap(col, n=1):
        s = SUM[:]
        if n == 1:
            return bass.AP(s.tensor, s.offset + col * DIM, [s.ap[0], [1, DIM]])
        return bass.AP(s.tensor, s.offset + col * DIM,
                       [s.ap[0], [DIM, n], [1, DIM]])

    def rs_ap(col, n=1):
        r = RS[:]
        if n == 1:
            return bass.AP(r.tensor, r.offset + col * DIM, [r.ap[0], [1, DIM]])
        return bass.AP(r.tensor, r.offset + col * DIM,
                       [r.ap[0], [DIM, n], [1, DIM]])

    def scatter(col):
        nc.gpsimd.indirect_dma_start(
            out=out_aps[col][:],
            out_offset=bass.IndirectOffsetOnAxis(
                ap=t32[:, NGCOL + col:NGCOL + col + 1], axis=0),
            in_=rs_ap(col), in_offset=None,
            bounds_check=NPATCH - 1, oob_is_err=False)

    # window w -> (AP, tf partition range) within the class tiles
    def mask_target(cls, q, w):
        if cls == "b" and w >= 4:
            return Plane(WPR, (q * 2 + (w - 4)) * DIM).ap(), slice(0, P)
        if cls == "c":
            return Plane(WC2, ((w // 4) * 4 + w % 4) * DIM).ap(), slice(0, P)
        # d (stacked): token w -> partition range 32*(w//4), plane w%4
        b = w // 4
        base = WD1[:]
        ap = bass.AP(base.tensor,
                     base.offset + 32 * b * base.ap[0][0] + (w % 4) * DIM,
                     [[base.ap[0][0], 32], [1, DIM]])
        return ap, slice(32 * b, 32 * b + 32)

    for mi, (cls, q, w) in enumerate(MASKS):
        ap, prange = mask_target(cls, q, w)
        o = NQ["a4"] + mi
        nc.scalar.activation(ap, ap, Ident,
                             scale=tf[prange, o:o + 1],
                             bias=tf[prange, o + len(MASKS):o + len(MASKS) + 1])

    mx = mybir.AluOpType.max
    mn = mybir.AluOpType.min
    add = mybir.AluOpType.add

    mx = mybir.AluOpType.max
    mn = mybir.AluOpType.min
    add = mybir.AluOpType.add

    # ---- class B: per-q sorts (q0's starts as soon as its gather lands),
    # then merge pair and sum jointly (J=2)
    emit_sort4(nc, WB2, TSB, 1, 4 * DIM, 4 * DIM)
    sb = [Plane(p.t, p.off, 4 * DIM) for p in
          emit_sort4(nc, WB2, TSB, 1, 4 * DIM, 4 * DIM,
                     xoff=4 * DIM, toff=4 * DIM)]
    sb = [Plane(p.t, p.off - 4 * DIM, 4 * DIM) for p in sb]
    p0 = Plane(WPR, 0, 2 * DIM)
    p1 = Plane(WPR, DIM, 2 * DIM)
    p_hi = Plane(M2, 0, 2 * DIM)
    p_lo = Plane(M2, DIM, 2 * DIM)
    nc.vector.tensor_tensor(p_hi.ap(2), p0.ap(2), p1.ap(2), op=mx)
    nc.vector.tensor_tensor(p_lo.ap(2), p0.ap(2), p1.ap(2), op=mn)
    nc.vector.tensor_tensor(p0.ap(2), sb[2].ap(2), p_lo.ap(2), op=mx)  # c2
    nc.vector.tensor_tensor(p1.ap(2), sb[3].ap(2), p_hi.ap(2), op=mx)  # c3
    emit_sum4(nc, sum_ap(COLS.index("bq0"), 2), [sb[0], sb[1], p0, p1], 2,
              p_hi, p_lo)
    nc.scalar.activation(rs_ap(COLS.index("bq0"), 2),
                         sum_ap(COLS.index("bq0"), 2), Copy, scale=0.25)
    scatter(COLS.index("bq0"))
    scatter(COLS.index("bq1"))

    # ---- class C: sort both blocks (J=2), merge, sum ----
    scp = emit_sort4(nc, WC2, TSC, 2, 4 * DIM, 4 * DIM)
    ca = [Plane(p.t, p.off) for p in scp]
    cb = [Plane(p.t, p.off + 4 * DIM) for p in scp]
    mc = [Plane(ME, w * DIM) for w in range(4)]
    emit_merge4(nc, mc, ca, cb)
    emit_sum4(nc, sum_ap(COLS.index("c")), mc, 1,
              Plane(ME, 4 * DIM), Plane(ME, 5 * DIM))
    nc.scalar.activation(rs_ap(COLS.index("c")), sum_ap(COLS.index("c")),
                         Copy, scale=0.25)
    scatter(COLS.index("c"))

    # ---- class D (stacked): sort J=1, rearrange blocks 1,2 planar ----
    emit_sort4_contig(nc, WD1, TSD, 1, 4 * DIM, 4 * DIM)
    nc.sync.dma_start(WDP[0:32, 0:4 * DIM], WD1[32:64, :])
    nc.sync.dma_start(WDP[0:32, 4 * DIM:8 * DIM], WD1[64:96, :])

    # ---- A classes (their 7 scatters overlap the D path) ----
    c0 = COLS.index("a4q0")
    a4 = [Plane(WA["a4"], w * DIM, 4 * DIM) for w in range(4)]
    emit_sum4(nc, sum_ap(c0, 2), a4, 2, Plane(ME, 0, 2 * DIM),
              Plane(ME, DIM, 2 * DIM))
    for q in range(2):
        nc.scalar.activation(rs_ap(c0 + q), sum_ap(c0 + q), Copy,
                             scale=tf[:, q:q + 1])
    fo = NQ["a4"] + 2 * len(MASKS)
    fix = bass.AP(tf[:].tensor, tf[:].offset + fo, [tf[:].ap[0], [1, DIM]])
    nc.vector.tensor_tensor(rs_ap(c0), rs_ap(c0), fix, op=add)
    scatter(c0)
    scatter(c0 + 1)

    c0 = COLS.index("a3q0")
    w0 = Plane(WA["a3"], 0, 3 * DIM)
    w1 = Plane(WA["a3"], DIM, 3 * DIM)
    w2 = Plane(WA["a3"], 2 * DIM, 3 * DIM)
    s0 = Plane(ME, 0, 2 * DIM)
    nc.vector.tensor_tensor(s0.ap(2), w0.ap(2), w1.ap(2), op=add)
    nc.vector.tensor_tensor(sum_ap(c0, 2), s0.ap(2), w2.ap(2), op=add)
    nc.scalar.activation(rs_ap(c0, 2), sum_ap(c0, 2), Copy,
                         scale=1.0 / 3.0)
    scatter(c0)
    scatter(c0 + 1)

    c0 = COLS.index("a2q0")
    w0 = Plane(WA["a2"], 0, 2 * DIM)
    w1 = Plane(WA["a2"], DIM, 2 * DIM)
    nc.vector.tensor_tensor(sum_ap(c0, 2), w0.ap(2), w1.ap(2), op=add)
    nc.scalar.activation(rs_ap(c0, 2), sum_ap(c0, 2), Copy, scale=0.5)
    scatter(c0)
    scatter(c0 + 1)

    c0 = COLS.index("a1")
    nc.scalar.activation(rs_ap(c0), Plane(WA["a1"], 0).ap(), Copy, scale=1.0)
    scatter(c0)

    # ---- class D path (planar, partitions 0..31) ----
    db0 = [PPlane(WD1, 0, 32, k * DIM) for k in range(4)]
    db1 = [PPlane(WDP, 0, 32, k * DIM) for k in range(4)]
    db2 = [PPlane(WDP, 0, 32, (4 + k) * DIM) for k in range(4)]
    me = [PPlane(ME, 0, 32, k * DIM) for k in range(4)]
    sc4 = [PPlane(M2, 0, 32, (4 + k) * DIM) for k in range(4)]
    emit_merge4(nc, me, db0, db1)
    e = emit_bitonic_sort4(nc, sc4, me)
    md = [PPlane(M2, 0, 32, k * DIM) for k in range(4)]
    emit_merge4(nc, md, e, db2)
    sd32 = SUM[0:32, :]
    dcol = COLS.index("d")
    emit_sum4(nc, bass.AP(sd32.tensor, sd32.offset + dcol * DIM,
                          [sd32.ap[0], [1, DIM]]), md, 1,
              PPlane(ME, 0, 32, 4 * DIM), PPlane(ME, 0, 32, 5 * DIM))
    nc.scalar.activation(rs_ap(dcol), sum_ap(dcol), Copy, scale=0.25)
    scatter(dcol)


def build_module(nrow, num_devices=8):
    nc = bacc.Bacc("TRN2", num_devices=num_devices, debug=False,
                   enable_asserts=False)
    with tile.TileContext(nc) as tc:
        with ExitStack() as ctx:
            build_kernel(ctx, tc, nrow)
    nc.compile()
    return nc


def _enable_axon_profiling():
    """Register the NTFF profile hook (the container image lacks
    antenv.axon_hooks; recreate it and wire the ctypes hook)."""
    import sys
    import types

    import antenv

    if 'antenv.axon_hooks' not in sys.modules:
        mod = types.ModuleType('antenv.axon_hooks')
        mod._hook = None
        mod.set_axon_ntff_profile_hook = lambda h: setattr(mod, '_hook', h)
        mod.get_axon_ntff_profile_hook = lambda: mod._hook
        sys.modules['antenv.axon_hooks'] = mod
        antenv.axon_hooks = mod
    from antenv import axon_hooks
    if axon_hooks.get_axon_ntff_profile_hook() is None:
        from trn_agent_boot.trn_boot import _ntff_profile_via_ctypes
        axon_hooks.set_axon_ntff_profile_hook(
            _ntff_profile_via_ctypes('/opt/axon/libaxon_pjrt.so'))
    import concourse.bass_utils as bu
    bu.upload_artifacts = lambda tmpdir: tmpdir


def kernel(h, patch_ids, max_num_patches, k, _profile=False):
    assert int(np.asarray(k)) == K
    assert int(np.asarray(max_num_patches)) == NPATCH
    nb = np.asarray(h).shape[0]
    if _profile:
        try:
            _enable_axon_profiling()
        except Exception as e:
            print(f"profiling setup failed ({e}); running without trace")
            _profile = False
    in_maps, nrow, srows = prepare(h, patch_ids)
    nc = build_module(nrow, num_devices=nb)
    res = run_bass_kernel_spmd(nc, in_maps, core_ids=list(range(nb)),
                               trace=_profile)
    out = np.empty((nb, NPATCH, DIM), np.float32)
    for b in range(nb):
        for col in range(NCOL):
            rows = srows[b][:, col]
            m = rows != OOB
            out[b, rows[m]] = res.results[b][f"out{col}"][rows[m]]
    if _profile:
        kernel.last_results = res
    return out
